# revision 2
# baseline (speedup 1.0000x reference)
"""Trainium2 Bass kernel for nn_CacheModel (retrieval_knn).

Computes out = log(exp(theta * (x/||x||) @ mem_keys) @ mem_vals) on 8
NeuronCores.  mem_keys is sharded column-wise and mem_vals row-wise over
the N_mem axis; each core computes its partial [1,1000] product, an
on-device AllReduce sums the partials, and each core takes the log.

Self-contained: hardcodes all shapes; imports only the system-installed
concourse stack + numpy.
"""

from contextlib import ExitStack

import numpy as np

import concourse.bass as bass
import concourse.tile as tile
from concourse import bacc, mybir

F32 = mybir.dt.float32
AF = mybir.ActivationFunctionType

# Problem shapes (full)
D_FEAT = 2048
N_MEM = 200000
N_CLASSES = 1000
THETA = 5.0
N_CORES = 8

# Per-core sharding: 25000 n-rows, zero-padded to 25088 = 196*128 = 49*512
N_SHARD = N_MEM // N_CORES          # 25000
WIN = 512                           # n-window width (one psum bank of f32)
N_PAD = 25088                       # 49 windows * 512
N_WINDOWS = N_PAD // WIN            # 49
CHUNKS_PER_WIN = WIN // 128         # 4
FEAT_CHUNKS = D_FEAT // 128         # 16
NC_HALF = N_CLASSES // 2            # 500 (<=512 moving-free-dim limit)


def build_kernel(
    num_devices: int = N_CORES,
    d_feat: int = D_FEAT,
    n_pad: int = N_PAD,
    n_classes: int = N_CLASSES,
    win: int = WIN,
    keys_bufs: int = 2,
    vals_bufs: int = 3,
):
    """Builds + compiles the per-core Bass program (SPMD: same program on
    every core; each core receives its own keys/vals shard)."""
    feat_chunks = d_feat // 128
    n_windows = n_pad // win
    chunks_per_win = win // 128
    nc_half = n_classes // 2
    n_chunks = n_pad // 128

    nc = bacc.Bacc(
        "TRN2",
        target_bir_lowering=False,
        debug=False,
        num_devices=num_devices,
    )

    x_d = nc.dram_tensor("x", [1, d_feat], F32, kind="ExternalInput").ap()
    keys_d = nc.dram_tensor("keys", [d_feat, n_pad], F32, kind="ExternalInput").ap()
    vals_d = nc.dram_tensor("vals", [n_pad, n_classes], F32, kind="ExternalInput").ap()
    out_d = nc.dram_tensor("out", [1, n_classes], F32, kind="ExternalOutput").ap()

    # DRAM views with the n axis split into 128-row chunks
    keys_v = keys_d.rearrange("(c p) n -> p c n", p=128)      # [128, feat_chunks, n_pad]
    vals_v = vals_d.rearrange("(q p) j -> p q j", p=128)      # [128, n_chunks, n_classes]

    with tile.TileContext(nc) as tc, ExitStack() as ctx:
        const = ctx.enter_context(tc.tile_pool(name="const", bufs=1))
        keys_pool = ctx.enter_context(tc.tile_pool(name="keys", bufs=keys_bufs))
        vals_pool = ctx.enter_context(tc.tile_pool(name="vals", bufs=vals_bufs))
        s_pool = ctx.enter_context(tc.tile_pool(name="s", bufs=4))
        st_pool = ctx.enter_context(tc.tile_pool(name="st", bufs=4))
        psum_s = ctx.enter_context(tc.tile_pool(name="psum_s", bufs=2, space="PSUM"))
        psum_t = ctx.enter_context(tc.tile_pool(name="psum_t", bufs=2, space="PSUM"))
        psum_p = ctx.enter_context(tc.tile_pool(name="psum_p", bufs=1, space="PSUM"))
        dram = ctx.enter_context(tc.tile_pool(name="dram", bufs=1, space="DRAM"))

        # ---- prologue: xt = x reshaped [128, feat_chunks]; scale = theta/||x||
        xt = const.tile([128, feat_chunks], F32)
        nc.sync.dma_start(out=xt[:], in_=x_d.rearrange("a (c p) -> p (a c)", p=128))

        ones = const.tile([128, 1], F32)
        nc.vector.memset(ones[:], 1.0)

        sq = const.tile([128, feat_chunks], F32)
        nc.vector.tensor_mul(sq[:], xt[:], xt[:])
        sums = const.tile([128, 1], F32)
        nc.vector.tensor_reduce(
            sums[:], sq[:], axis=mybir.AxisListType.X, op=mybir.AluOpType.add
        )
        nrm2_ps = psum_t.tile([1, 1], F32, tag="nrm2")
        nc.tensor.matmul(nrm2_ps[:], lhsT=ones[:], rhs=sums[:], start=True, stop=True)
        nrm = const.tile([1, 1], F32)
        nc.scalar.sqrt(nrm[:], nrm2_ps[:])
        inv = const.tile([1, 1], F32)
        nc.vector.reciprocal(inv[:], nrm[:])
        scale = const.tile([1, 1], F32)
        nc.vector.tensor_scalar_mul(scale[:], inv[:], THETA)

        # ---- persistent [1, n_classes] accumulator (2 psum banks)
        pp_a = psum_p.tile([1, nc_half], F32, tag="pp_a")
        pp_b = psum_p.tile([1, nc_half], F32, tag="pp_b")

        for w in range(n_windows):
            kt = keys_pool.tile([128, feat_chunks, win], F32)
            nc.sync.dma_start(out=kt[:], in_=keys_v[:, :, w * win:(w + 1) * win])
            vt = vals_pool.tile([128, chunks_per_win, n_classes], F32)
            nc.sync.dma_start(
                out=vt[:],
                in_=vals_v[:, w * chunks_per_win:(w + 1) * chunks_per_win, :],
            )

            # stage 1: u[1, win] = sum_c xt[:,c].T @ kt[:,c,:]
            ps_s = psum_s.tile([1, win], F32)
            for c in range(feat_chunks):
                nc.tensor.matmul(
                    ps_s[:],
                    lhsT=xt[:, c:c + 1],
                    rhs=kt[:, c, :],
                    start=(c == 0),
                    stop=(c == feat_chunks - 1),
                )

            # s = exp(scale * u)   (ACT reads PSUM, writes SBUF)
            s_exp = s_pool.tile([1, win], F32)
            nc.scalar.activation(s_exp[:], ps_s[:], AF.Exp, scale=scale[:])

            # transpose s into partition-major [128, chunks_per_win] via K=1 matmuls
            st = st_pool.tile([128, chunks_per_win], F32)
            for q in range(chunks_per_win):
                ps_t = psum_t.tile([128, 1], F32, tag="ps_t")
                nc.tensor.matmul(
                    ps_t[:],
                    lhsT=s_exp[:, q * 128:(q + 1) * 128],
                    rhs=ones[0:1, 0:1],
                    start=True,
                    stop=True,
                )
                nc.vector.tensor_copy(st[:, q:q + 1], ps_t[:])

            # stage 2: pp += st[:,q].T @ vt[:,q,:]   (accumulates across windows)
            for q in range(chunks_per_win):
                gc = w * chunks_per_win + q
                first = gc == 0
                last = gc == n_chunks - 1
                nc.tensor.matmul(
                    pp_a[:],
                    lhsT=st[:, q:q + 1],
                    rhs=vt[:, q, 0:nc_half],
                    start=first,
                    stop=last,
                )
                nc.tensor.matmul(
                    pp_b[:],
                    lhsT=st[:, q:q + 1],
                    rhs=vt[:, q, nc_half:n_classes],
                    start=first,
                    stop=last,
                )

        # ---- tail: partial -> DRAM, AllReduce, log, output
        p_sb = const.tile([1, n_classes], F32)
        nc.vector.tensor_copy(p_sb[:, 0:nc_half], pp_a[:])
        nc.vector.tensor_copy(p_sb[:, nc_half:n_classes], pp_b[:])

        partial = dram.tile([1, n_classes], F32)
        reduced = dram.tile([1, n_classes], F32)
        nc.gpsimd.dma_start(partial[:], p_sb[:])
        nc.gpsimd.collective_compute(
            "AllReduce",
            mybir.AluOpType.add,
            replica_groups=[list(range(num_devices))],
            ins=[partial.opt()],
            outs=[reduced.opt()],
        )
        red_sb = const.tile([1, n_classes], F32)
        nc.sync.dma_start(red_sb[:], reduced[:])
        logp = const.tile([1, n_classes], F32)
        nc.scalar.activation(logp[:], red_sb[:], AF.Ln)
        nc.sync.dma_start(out_d[:], logp[:])

    nc.compile()
    return nc


_NC_CACHE: dict = {}


def _get_nc():
    if "nc" not in _NC_CACHE:
        _NC_CACHE["nc"] = build_kernel()
    return _NC_CACHE["nc"]


def _shard_inputs(x, mem_keys, mem_vals):
    x = np.ascontiguousarray(np.asarray(x, dtype=np.float32))
    in_maps = []
    for i in range(N_CORES):
        lo, hi = i * N_SHARD, (i + 1) * N_SHARD
        keys_shard = np.zeros((D_FEAT, N_PAD), dtype=np.float32)
        keys_shard[:, :N_SHARD] = mem_keys[:, lo:hi]
        vals_shard = np.zeros((N_PAD, N_CLASSES), dtype=np.float32)
        vals_shard[:N_SHARD, :] = mem_vals[lo:hi, :]
        in_maps.append({"x": x, "keys": keys_shard, "vals": vals_shard})
    return in_maps


def run(x, mem_keys, mem_vals, trace: bool = False):
    """Runs the SPMD kernel; returns (output [1, N_CLASSES], BassKernelResults)."""
    from concourse.bass_utils import run_bass_kernel_spmd

    nc = _get_nc()
    in_maps = _shard_inputs(x, mem_keys, mem_vals)
    res = run_bass_kernel_spmd(nc, in_maps, list(range(N_CORES)), trace=trace)
    out = np.asarray(res.results[0]["out"], dtype=np.float32).reshape(1, N_CLASSES)
    return out, res


def kernel(x, mem_keys, mem_vals):
    out, _ = run(x, mem_keys, mem_vals, trace=False)
    return out
